# revision 1
# baseline (speedup 1.0000x reference)
"""Causal self-attention (B=2, T=2048, D=1024, H=16, hd=64) on 8 TRN2 cores.

Sharding: 2 batches x 4 head-groups (4 heads each). Each core computes the
full pipeline for its (batch, head-group): qkv projection (transposed
layout), causal attention, and its partial output projection. The host sums
the 4 per-batch partials (tensor-parallel reduce) and adds bproj.

Device-side layout notes:
 - x is passed pre-transposed (xT [D, T]) so the qkv projection can contract
   over D on the partition dimension.
 - Scores are computed transposed (St = k @ qT, [k_tok, q_tok]) so softmax's
   exp feeds straight into att@v as the moving operand without transposes.
 - Softmax has no max-subtraction (scores are O(6) here, exp is safe) and the
   denominator is produced by augmenting v with a ones column (M=65 matmul).
 - The 1/sqrt(hd) scale is folded into Wq/bq on the host.
"""

import sys

sys.path.insert(0, "/opt/trn_rl_repo")

import numpy as np
from collections import deque

B, T, D = 2, 2048, 1024
N_HEAD = 16
HD = 64  # head dim
HPC = 4  # heads per core
N_CORES = 8

P = 128
NJ = 512  # q-slice width
JT = T // NJ  # 4 q-slices
KT = D // P  # 8 contraction tiles for qkv
MT = 6  # qkv m-tiles: 2 q, 2 k, 2 v (128 dims each)
NQKV = MT * P  # 768
IT = T // P  # 16 k-token tiles

_CACHE = {}


def _build():
    import concourse.bass as bass  # noqa: F401
    import concourse.mybir as mybir
    import concourse.tile as tile
    from concourse import bacc

    F32 = mybir.dt.float32
    F32R = mybir.dt.float32r
    AF = mybir.ActivationFunctionType

    nc = bacc.Bacc(None, target_bir_lowering=False)
    xT_d = nc.dram_tensor("xT", [D, T], F32R, kind="ExternalInput")
    wqkv_d = nc.dram_tensor("wqkv", [D, NQKV], F32R, kind="ExternalInput")
    bqkv_d = nc.dram_tensor("bqkv2", [P, MT], F32, kind="ExternalInput")
    wproj_d = nc.dram_tensor("wproj", [HD, HPC * D], F32R, kind="ExternalInput")
    masks_d = nc.dram_tensor("masks", [P, NJ], F32R, kind="ExternalInput")
    ident_d = nc.dram_tensor("ident", [P, P], F32R, kind="ExternalInput")
    out_d = nc.dram_tensor("out", [T, D], F32, kind="ExternalOutput")

    with tile.TileContext(nc) as tc:
        with (
            tc.tile_pool(name="const", bufs=1) as const,
            tc.tile_pool(name="xp", bufs=2) as xp,
            tc.tile_pool(name="stps", bufs=2, space="PSUM") as stps,
            tc.tile_pool(name="yps", bufs=2, space="PSUM") as yps,
            tc.tile_pool(name="expp", bufs=6) as expp,
            tc.tile_pool(name="recp", bufs=2) as recp,
            tc.tile_pool(name="bcp", bufs=2) as bcp,
            tc.tile_pool(name="outp", bufs=2) as outp,
        ):
            w_sb = const.tile([P, KT, NQKV], F32R)
            bias_sb = const.tile([P, MT], F32)
            wp_sb = const.tile([HD, HPC * D], F32R)
            masks_sb = const.tile([P, NJ], F32R)
            ident = const.tile([P, P], F32R)
            qkvT_sb = const.tile([P, MT, T], F32R)
            vnat_sb = const.tile([P, 2, IT, 130], F32R)
            yt_sb = const.tile([HD, HPC, T], F32R)

            w_r = wqkv_d.rearrange("(kt p) n -> p kt n", p=P)
            for k in range(KT):
                nc.sync.dma_start(w_sb[:, k, :], w_r[:, k, :])
            nc.sync.dma_start(bias_sb[:], bqkv_d[:])
            nc.sync.dma_start(wp_sb[:], wproj_d[:])
            nc.sync.dma_start(masks_sb[:], masks_d[:])
            nc.sync.dma_start(ident[:], ident_d[:])

            xT_r = xT_d.rearrange("(kt p) t -> p kt t", p=P)

            # ---- Stage 1+2 as schedulable units ------------------------
            # qkv projection groups and v-transposes for q-slice j+1 are
            # interleaved into attention slice j's loop as dense, wait-free
            # PE filler (keeps the PE activity monitor warm).
            xts = {}

            def emit_xt(j):
                xt = xp.tile([P, KT, NJ], F32R, tag="xt", name=f"xt{j}")
                for k in range(KT):
                    nc.sync.dma_start(
                        xt[:, k, :], xT_r[:, k, j * NJ : (j + 1) * NJ]
                    )
                xts[j] = xt

            def emit_qkv_group(j, m):
                ps = stps.tile([P, NJ], F32, tag="st", name=f"qkvps{j}_{m}")
                for k in range(KT):
                    nc.tensor.matmul(
                        ps[:],
                        w_sb[:, k, m * P : (m + 1) * P],
                        xts[j][:, k, :],
                        start=(k == 0),
                        stop=(k == KT - 1),
                    )
                nc.scalar.activation(
                    qkvT_sb[:, m, j * NJ : (j + 1) * NJ],
                    ps[:],
                    AF.Identity,
                    bias=bias_sb[:, m : m + 1],
                )

            def emit_vt(h2, ii):
                pt = stps.tile([P, NJ], F32R, tag="st", name=f"vt{h2}_{ii}")
                nc.tensor.transpose(
                    pt[:, 0:P], qkvT_sb[:, 4 + h2, ii * P : (ii + 1) * P], ident[:]
                )
                nc.vector.tensor_copy(vnat_sb[:, h2, ii, 0:HD], pt[:, 0:HD])
                nc.vector.tensor_copy(vnat_sb[:, h2, ii, 65 : 65 + HD], pt[:, HD:P])

            fill_q = deque()

            def push_fill(j):
                for m in range(MT):
                    fill_q.append(("qkv", j, m))
                for h2 in range(2):
                    for ii in range(4 * j, 4 * j + 4):
                        fill_q.append(("vt", j, h2, ii))

            def emit_fill(item):
                if item[0] == "qkv":
                    emit_qkv_group(item[1], item[2])
                else:
                    emit_vt(item[2], item[3])

            for h2 in range(2):
                nc.gpsimd.memset(vnat_sb[:, h2, :, :].bitcast(F32), 1.0)
            emit_xt(0)
            push_fill(0)
            while fill_q:
                emit_fill(fill_q.popleft())

            # ---- Stage 3: attention per head-pair ----------------------
            # Software-pipelined: St(i) is issued before Y(i-1) so ACT's
            # exp(i-1) overlaps the PE's St(i); normalization of slice (hp,j)
            # is deferred into slice (hp,j)+1's loop so the reciprocal's
            # latency hides behind matmul work.

            def emit_recip(pend):
                hp_, j_, y2_ = pend
                recs = []
                for par in range(2):
                    rec = recp.tile([P, NJ], F32R, tag="rec")
                    with nc.allow_low_precision(reason="f32r is 4-byte"):
                        nc.vector.reciprocal(rec[64:65, :], y2_[64:65, par, :])
                    recs.append(rec)
                return (hp_, j_, y2_, recs)

            def emit_norm(pend):
                hp_, j_, y2_, recs = pend
                for par in range(2):
                    rec = recs[par]
                    # broadcast row 64 across 64 partitions via a K=1
                    # matmul; masks_sb[64, 448:512] is an all-ones run.
                    bc_ps = stps.tile([P, 2, NJ], F32, tag="st")
                    nc.tensor.matmul(
                        bc_ps[0:HD, 0, :],
                        masks_sb[64:65, 448:512],
                        rec[64:65, :],
                        start=True,
                        stop=True,
                    )
                    bc = bcp.tile([HD, NJ], F32)
                    nc.scalar.activation(bc[:, :], bc_ps[0:HD, 0, :], AF.Copy)
                    nc.vector.tensor_mul(
                        yt_sb[:, 2 * hp_ + par, j_ * NJ : (j_ + 1) * NJ],
                        y2_[0:HD, par, :],
                        bc[:, :],
                    )

            def emit_y(rec):
                hp_, pi, pexp, y2_, last, j_ = rec
                nc.tensor.matmul(
                    y2_[0:65, 0, :],
                    vnat_sb[:, hp_, pi, 0:65],
                    pexp[:, 0, :],
                    start=(pi == 0),
                    stop=last,
                )
                nc.tensor.matmul(
                    y2_[0:65, 1, :],
                    vnat_sb[:, hp_, pi, 65:130],
                    pexp[:, 1, :],
                    start=(pi == 0),
                    stop=last,
                )

            def emit_proj(item):
                qm_i, n = item
                po = stps.tile([P, 2, NJ], F32, tag="st")
                for h in range(HPC):
                    nc.tensor.matmul(
                        po[:, 0, :],
                        yt_sb[:, h, qm_i * P : (qm_i + 1) * P],
                        wp_sb[:, h * D + n * NJ : h * D + (n + 1) * NJ].bitcast(F32R),
                        start=(h == 0),
                        stop=(h == HPC - 1),
                    )
                ot = outp.tile([P, NJ], F32)
                nc.vector.tensor_copy(ot[:], po[:, 0, :])
                nc.sync.dma_start(
                    out_d[qm_i * P : (qm_i + 1) * P, n * NJ : (n + 1) * NJ], ot[:]
                )

            # Global software pipeline over slices (j outer, hp inner):
            # y matmuls trail the St/exp stream by DEPTH iterations and spill
            # across slice boundaries; each slice's normalization runs inside
            # a later slice's loop; proj groups for q-slice j are spread one
            # per iteration once both head-pairs of j are normalized.
            DEPTH = 4
            y_q = deque()  # (hp, i, exp2, y2holder, last, j)
            norm_q = deque()  # (hp, j, y2, recs)
            proj_items = deque()  # (qm_i, n)
            y2_map = {}  # (hp, j) -> y2 tile, allocated lazily at first y

            def emit_y2(rec):
                hp_, pi, pexp, _, last, j_ = rec
                if pi == 0:
                    y2_map[(hp_, j_)] = yps.tile(
                        [P, 2, NJ], F32, tag="y", name=f"y2_{hp_}_{j_}"
                    )
                emit_y((hp_, pi, pexp, y2_map[(hp_, j_)], last, j_))
                if last:
                    norm_q.append(
                        emit_recip((hp_, j_, y2_map.pop((hp_, j_))))
                    )

            def pump(i):
                if i >= 2 and norm_q:
                    pend = norm_q.popleft()
                    emit_norm(pend)
                    if pend[0] == 1:  # second head-pair of slice j done
                        for qq in range(4 * pend[1], 4 * pend[1] + 4):
                            proj_items.append((qq, 0))
                            proj_items.append((qq, 1))
                elif fill_q:
                    emit_fill(fill_q.popleft())
                    if len(fill_q) > 8:
                        emit_fill(fill_q.popleft())
                elif i >= 2 and proj_items:
                    emit_proj(proj_items.popleft())

            for j in range(JT):
                if j + 1 < JT:
                    emit_xt(j + 1)
                    push_fill(j + 1)
                n_i = 4 * j + 4
                for i in range(n_i):
                    for hp in range(2):
                        if len(y_q) > DEPTH:
                            emit_y2(y_q.popleft())
                        qm, km = hp, 2 + hp
                        st2 = stps.tile([P, 2, NJ], F32, tag="st")
                        nc.tensor.matmul(
                            st2[:, 0, :],
                            qkvT_sb[0:HD, km, i * P : (i + 1) * P],
                            qkvT_sb[0:HD, qm, j * NJ : (j + 1) * NJ],
                            start=True,
                            stop=True,
                            tile_position=(0, 0),
                        )
                        nc.tensor.matmul(
                            st2[:, 1, :],
                            qkvT_sb[HD:P, km, i * P : (i + 1) * P],
                            qkvT_sb[HD:P, qm, j * NJ : (j + 1) * NJ],
                            start=True,
                            stop=True,
                            tile_position=(64, 0),
                        )
                        exp2 = expp.tile([P, 2, NJ], F32R, tag="exp")
                        r = i - 4 * j
                        if r < 0:
                            nc.scalar.activation(exp2[:], st2[:], AF.Exp)
                        else:
                            # diag block: cols [0, 128r) are fully above the
                            # causal line -> zero; cols [128r, 128r+128) are
                            # triangular; the rest is fully kept.
                            c0 = P * r
                            if c0 > 0:
                                nc.gpsimd.memset(
                                    exp2[:, :, 0:c0].bitcast(F32), 0.0
                                )
                            nc.scalar.activation(
                                exp2[:, :, c0:NJ], st2[:, :, c0:NJ], AF.Exp
                            )
                            for par in range(2):
                                nc.vector.tensor_mul(
                                    exp2[:, par, c0 : c0 + P],
                                    exp2[:, par, c0 : c0 + P],
                                    masks_sb[:, 0:P],
                                )
                        y_q.append((hp, i, exp2, None, i == n_i - 1, j))
                        pump(i)
                # slice j+1's qkv/vT must be complete before its St reads
                while fill_q:
                    emit_fill(fill_q.popleft())

            while y_q:
                emit_y2(y_q.popleft())
            while norm_q:
                pend = norm_q.popleft()
                emit_norm(pend)
                if pend[0] == 1:
                    for qq in range(4 * pend[1], 4 * pend[1] + 4):
                        proj_items.append((qq, 0))
                        proj_items.append((qq, 1))
            while proj_items:
                emit_proj(proj_items.popleft())

    nc.compile()
    return nc


def _prep_inputs(x, Wqkv, bqkv, Wproj):
    """Per-core input maps. Core c -> batch c//4, heads 4*(c%4) .. +4."""
    scale = np.float32(1.0 / np.sqrt(HD))
    pp = np.arange(P)[:, None]
    ff = np.arange(NJ)[None, :]
    masks = (ff >= pp).astype(np.float32)

    in_maps = []
    for c in range(N_CORES):
        b, g = divmod(c, HPC)
        cs = slice(256 * g, 256 * g + 256)
        wq = Wqkv[:, 0 * D :][:, cs] * scale
        wk = Wqkv[:, 1 * D : 2 * D][:, cs]
        wv = Wqkv[:, 2 * D : 3 * D][:, cs]
        wqkv_c = np.ascontiguousarray(np.concatenate([wq, wk, wv], axis=1), np.float32)
        bq = bqkv[0 * D :][cs] * scale
        bk = bqkv[1 * D : 2 * D][cs]
        bv = bqkv[2 * D : 3 * D][cs]
        bqkv_c = np.concatenate([bq, bk, bv]).reshape(MT, P).T
        wproj_c = np.concatenate(
            [Wproj[256 * g + HD * h : 256 * g + HD * (h + 1), :] for h in range(HPC)],
            axis=1,
        )
        in_maps.append(
            {
                "xT": np.ascontiguousarray(x[b].T, np.float32),
                "wqkv": wqkv_c,
                "bqkv2": np.ascontiguousarray(bqkv_c, np.float32),
                "wproj": np.ascontiguousarray(wproj_c, np.float32),
                "masks": masks,
                "ident": np.eye(P, dtype=np.float32),
            }
        )
    return in_maps


def kernel(x, Wqkv, bqkv, Wproj, bproj, _trace=False, _trace_out=None):
    from concourse.bass_utils import run_bass_kernel_spmd

    if "nc" not in _CACHE:
        _CACHE["nc"] = _build()
    nc = _CACHE["nc"]

    x = np.asarray(x, np.float32)
    Wqkv = np.asarray(Wqkv, np.float32)
    bqkv = np.asarray(bqkv, np.float32)
    Wproj = np.asarray(Wproj, np.float32)
    bproj = np.asarray(bproj, np.float32)

    in_maps = _prep_inputs(x, Wqkv, bqkv, Wproj)
    res = run_bass_kernel_spmd(
        nc, in_maps, core_ids=list(range(N_CORES)), trace=_trace
    )
    if _trace_out is not None:
        _trace_out.append(res)

    out = np.empty((B, T, D), np.float32)
    for b in range(B):
        acc = res.results[HPC * b]["out"].astype(np.float32)
        for g in range(1, HPC):
            acc = acc + res.results[HPC * b + g]["out"]
        out[b] = acc + bproj[None, :]
    return out



# revision 4
# speedup vs baseline: 1.4010x; 1.4010x over previous
"""Causal self-attention (B=2, T=2048, D=1024, H=16, hd=64) on 8 TRN2 cores.

Sharding: 2 batches x 4 head-groups (4 heads each). Each core computes the
full pipeline for its (batch, head-group); the host sums the 4 per-batch
partials (tensor-parallel reduce) and adds bproj.

v2 design (vs the f32r baseline):
 - bf16 everywhere on the PE (PSUM accumulation stays f32): q/k/v weights,
   activations, exp, proj. Validated ~3e-3 rel err vs the 2e-2 gate.
 - v is produced directly in natural [token, hd] layout (stationary = xT
   block, moving = Wv columns), with the +bv handled by a K=1 ones-row
   matmul. No PE transposes.
 - Attention runs head-pair-sequential per q-slice so only one y-accumulator
   psum tile is live at a time; its pool slot rotates per head-pair, which
   removes the slice-boundary stalls the baseline had.
 - Causal diagonal blocks slice the St/exp/y ranges instead of memsetting
   masked regions (bf16 matmuls run 1 cycle/row at any width).
 - Softmax normalization: DVE copies the two denominator rows out of PSUM,
   a K=1 bf16 matmul broadcasts them across 64 partitions, one
   reciprocal_approx_fast inverts both broadcasts, and two DVE muls write
   the normalized yT with the second head shifted to partitions 64:127 so
   the output projection contracts K=128 (two matmuls instead of four).
"""

import sys

sys.path.insert(0, "/opt/trn_rl_repo")

import numpy as np
import ml_dtypes
from collections import deque

B, T, D = 2, 2048, 1024
N_HEAD = 16
HD = 64  # head dim
HPC = 4  # heads per core
N_CORES = 8

P = 128
NJ = 512  # q-slice width
JT = T // NJ  # 4 q-slices
KT = D // P  # 8 contraction tiles
IT = T // P  # 16 token tiles

_CACHE = {}


def _build():
    import concourse.bass as bass  # noqa: F401
    import concourse.mybir as mybir
    import concourse.tile as tile
    from concourse import bacc

    F32 = mybir.dt.float32
    BF16 = mybir.dt.bfloat16
    AF = mybir.ActivationFunctionType

    nc = bacc.Bacc(None, target_bir_lowering=False)
    xT_d = nc.dram_tensor("xT", [D, T], BF16, kind="ExternalInput")
    wqk_d = nc.dram_tensor("wqk", [D, 4 * P], BF16, kind="ExternalInput")
    bqk_d = nc.dram_tensor("bqk", [P, 4], F32, kind="ExternalInput")
    wv_d = nc.dram_tensor("wv", [D, 4 * 65], BF16, kind="ExternalInput")
    bv_d = nc.dram_tensor("bv", [1, 4 * 65], BF16, kind="ExternalInput")
    wp_d = nc.dram_tensor("wp", [2 * P, D], BF16, kind="ExternalInput")
    masks_d = nc.dram_tensor("masks", [P, 2 * P], BF16, kind="ExternalInput")
    out_d = nc.dram_tensor("out", [T, D], F32, kind="ExternalOutput")

    with tile.TileContext(nc) as tc:
        with (
            tc.tile_pool(name="const", bufs=1) as const,
            tc.tile_pool(name="stp", bufs=2, space="PSUM") as stp,
            tc.tile_pool(name="yp", bufs=2, space="PSUM") as yp,
            tc.tile_pool(name="expp", bufs=6) as expp,
            tc.tile_pool(name="denp", bufs=2) as denp,
            tc.tile_pool(name="recp", bufs=2) as recp,
            tc.tile_pool(name="outp", bufs=2) as outp,
        ):
            w_sb = const.tile([P, KT, 4 * P], BF16)
            bqk_sb = const.tile([P, 4], F32)
            wv_sb = const.tile([P, KT, 4, 65], BF16)
            bv_sb = const.tile([P, 4, 65], BF16)
            wp_sb = const.tile([P, 2, D], BF16)
            masks_sb = const.tile([P, 2, P], BF16)
            ones_sb = const.tile([P, P], BF16)
            xt_sb = const.tile([P, KT, T], BF16)
            qkvT = const.tile([P, 4, T], BF16)
            vnat = const.tile([P, IT, 4, 65], BF16)
            yt2 = const.tile([P, 2, T], BF16)

            nc.gpsimd.memset(ones_sb[:], 1.0)
            nc.gpsimd.memset(vnat[:], 1.0)

            xT_r = xT_d.rearrange("(kt p) t -> p kt t", p=P)
            wqk_r = wqk_d.rearrange("(kt p) n -> p kt n", p=P)
            wv_r = wv_d.rearrange("(kt p) n -> p kt n", p=P)
            for k in range(KT):
                nc.sync.dma_start(xt_sb[:, k, 0:NJ], xT_r[:, k, 0:NJ])
                nc.sync.dma_start(w_sb[:, k, :], wqk_r[:, k, :])
                nc.sync.dma_start(wv_sb[:, k, :, :], wv_r[:, k, :])
            nc.sync.dma_start(bqk_sb[:], bqk_d[:])
            nc.sync.dma_start(bv_sb[0:1, :, :], bv_d[:])
            nc.sync.dma_start(
                wp_sb[:], wp_d.rearrange("(hp p) d -> p hp d", p=P)
            )
            nc.sync.dma_start(masks_sb[:], masks_d.rearrange("p (a b) -> p a b", a=2))

            def emit_xt(j):
                for k in range(KT):
                    nc.sync.dma_start(
                        xt_sb[:, k, j * NJ : (j + 1) * NJ],
                        xT_r[:, k, j * NJ : (j + 1) * NJ],
                    )

            # ---- fills: qkv q/k groups + natural-layout v tiles ------------
            def emit_qkv(j, m):
                ps = stp.tile([P, NJ], F32, tag="st", name=f"qkvps{j}_{m}")
                for k in range(KT):
                    nc.tensor.matmul(
                        ps[:],
                        w_sb[:, k, m * P : (m + 1) * P],
                        xt_sb[:, k, j * NJ : (j + 1) * NJ],
                        start=(k == 0),
                        stop=(k == KT - 1),
                    )
                with nc.allow_low_precision(reason="bf16 activations"):
                    nc.vector.tensor_scalar_add(
                        qkvT[:, m, j * NJ : (j + 1) * NJ], ps[:], bqk_sb[:, m : m + 1]
                    )

            def emit_v(ii):
                ps = stp.tile([P, 4, 65], F32, tag="st", name=f"vps{ii}")
                for k in range(KT):
                    nc.tensor.matmul(
                        ps[:],
                        xt_sb[:, k, ii * P : (ii + 1) * P],
                        wv_sb[:, k, :, :],
                        start=(k == 0),
                        stop=False,
                    )
                nc.tensor.matmul(
                    ps[:],
                    ones_sb[0:1, 0:P],
                    bv_sb[0:1, :, :],
                    start=False,
                    stop=True,
                )
                with nc.allow_low_precision(reason="bf16 activations"):
                    nc.vector.tensor_copy(vnat[:, ii, :, 0:HD], ps[:, :, 0:HD])

            fill_q = deque()

            def push_fill(j):
                fill_q.append(("qkv", j, 0))
                fill_q.append(("qkv", j, 2))
                for ii in range(4 * j, 4 * j + 4):
                    fill_q.append(("v", ii))
                fill_q.append(("qkv", j, 1))
                fill_q.append(("qkv", j, 3))

            def emit_fill(item):
                if item[0] == "qkv":
                    emit_qkv(item[1], item[2])
                else:
                    emit_v(item[1])

            proj_q = deque()

            def emit_proj(item):
                # po lives in the "st" ring: every st-slot tenant's releasing
                # reader is emitted in the same emit_* call, so a PE matmul
                # here can never wait on a not-yet-emitted instruction.
                qm, n = item
                po = stp.tile([P, NJ], F32, tag="st", name=f"po{qm}_{n}")
                for hp in range(2):
                    nc.tensor.matmul(
                        po[:],
                        yt2[:, hp, qm * P : (qm + 1) * P],
                        wp_sb[:, hp, n * NJ : (n + 1) * NJ],
                        start=(hp == 0),
                        stop=(hp == 1),
                    )
                ot = outp.tile([P, NJ], F32, tag="ot")
                nc.vector.tensor_copy(ot[:], po[:])
                nc.sync.dma_start(
                    out_d[qm * P : (qm + 1) * P, n * NJ : (n + 1) * NJ], ot[:]
                )

            def pump():
                if fill_q:
                    emit_fill(fill_q.popleft())
                elif proj_q:
                    emit_proj(proj_q.popleft())

            # ---- attention -------------------------------------------------
            def emit_st_exp(j, hp, i):
                r = i - 4 * j
                c0 = max(0, P * r)
                st = stp.tile([P, 2, NJ], F32, tag="st", name=f"st{j}_{hp}_{i}")
                for par in range(2):
                    rows = slice(HD * par, HD * par + HD)
                    nc.tensor.matmul(
                        st[:, par, c0:NJ],
                        qkvT[rows, 2 + hp, i * P : (i + 1) * P],
                        qkvT[rows, hp, j * NJ + c0 : (j + 1) * NJ],
                        start=True,
                        stop=True,
                        tile_position=(HD * par, 0),
                    )
                exp2 = expp.tile([P, 2, NJ], BF16, tag="exp")
                nc.scalar.activation(exp2[:, :, c0:NJ], st[:, :, c0:NJ], AF.Exp)
                if r >= 0:
                    nc.gpsimd.tensor_mul(
                        exp2[:, :, c0 : c0 + P],
                        exp2[:, :, c0 : c0 + P],
                        masks_sb[:],
                    )
                return exp2

            def emit_y(j, hp, i, exp2, y2, last):
                r = i - 4 * j
                c0 = max(0, P * r)
                for par in range(2):
                    nc.tensor.matmul(
                        y2[0:65, par, c0:NJ],
                        vnat[:, i, 2 * hp + par, :],
                        exp2[:, par, c0:NJ],
                        start=(i == 0),
                        stop=last,
                    )

            def emit_norm(hp, j, y2):
                den = denp.tile([P, 2, NJ], BF16, tag="den")
                with nc.allow_low_precision(reason="bf16 denominator"):
                    nc.vector.tensor_copy(den[HD:65, :, :], y2[HD:65, :, :])
                bc = stp.tile([P, 2, NJ], F32, tag="st", name=f"bc{hp}_{j}")
                for par in range(2):
                    nc.tensor.matmul(
                        bc[0:HD, par, :],
                        ones_sb[HD : HD + 1, 0:HD],
                        den[HD : HD + 1, par, :],
                        start=True,
                        stop=True,
                    )
                rec = recp.tile([P, 2, NJ], F32, tag="rec")
                nc.vector.reciprocal_approx_fast(rec[0:HD, :, :], bc[0:HD, :, :])
                with nc.allow_low_precision(reason="bf16 yT"):
                    for par in range(2):
                        nc.vector.tensor_mul(
                            yt2[
                                HD * par : HD * par + HD,
                                hp,
                                j * NJ : (j + 1) * NJ,
                            ],
                            y2[0:HD, par, :],
                            rec[0:HD, par, :],
                        )

            DEPTH = 2
            y_q = deque()  # (hp, i, exp2, last, j)

            push_fill(0)
            while fill_q:
                emit_fill(fill_q.popleft())

            for j in range(JT):
                if j + 1 < JT:
                    emit_xt(j + 1)
                    push_fill(j + 1)
                n_i = 4 * j + 4
                for hp in range(2):
                    y2 = yp.tile([P, 2, NJ], F32, tag="y", name=f"y2_{hp}_{j}")
                    for i in range(n_i):
                        if len(y_q) > DEPTH:
                            rec_ = y_q.popleft()
                            emit_y(j, rec_[0], rec_[1], rec_[2], y2, rec_[3])
                        exp2 = emit_st_exp(j, hp, i)
                        y_q.append((hp, i, exp2, i == n_i - 1))
                        pump()
                    while y_q:
                        rec_ = y_q.popleft()
                        emit_y(j, rec_[0], rec_[1], rec_[2], y2, rec_[3])
                    emit_norm(hp, j, y2)
                for qm in range(4 * j, 4 * j + 4):
                    proj_q.append((qm, 0))
                    proj_q.append((qm, 1))

            while fill_q:
                emit_fill(fill_q.popleft())
            while proj_q:
                emit_proj(proj_q.popleft())

    nc.compile()
    return nc


def _prep_inputs(x, Wqkv, bqkv, Wproj):
    """Per-core input maps. Core c -> batch c//4, heads 4*(c%4) .. +4."""
    BF = ml_dtypes.bfloat16
    scale = np.float32(1.0 / np.sqrt(HD))
    pp = np.arange(P)[:, None]
    ff = np.arange(P)[None, :]
    tri = (ff >= pp).astype(np.float32)
    masks = np.concatenate([tri, tri], axis=1)

    in_maps = []
    for c in range(N_CORES):
        b, g = divmod(c, HPC)
        cs = slice(256 * g, 256 * g + 256)
        wq = Wqkv[:, 0 * D :][:, cs] * scale
        wk = Wqkv[:, 1 * D : 2 * D][:, cs]
        wv = Wqkv[:, 2 * D : 3 * D][:, cs]
        wqk_c = np.ascontiguousarray(np.concatenate([wq, wk], axis=1))
        bq = bqkv[0 * D :][cs] * scale
        bk = bqkv[1 * D : 2 * D][cs]
        bv = bqkv[2 * D : 3 * D][cs]
        bqk_c = np.concatenate([bq, bk]).reshape(4, P).T
        wv_c = np.zeros((D, 4 * 65), np.float32)
        bv_c = np.zeros((1, 4 * 65), np.float32)
        for h in range(4):
            wv_c[:, 65 * h : 65 * h + HD] = wv[:, HD * h : HD * (h + 1)]
            bv_c[0, 65 * h : 65 * h + HD] = bv[HD * h : HD * (h + 1)]
        wp_c = Wproj[256 * g : 256 * (g + 1), :]
        in_maps.append(
            {
                "xT": np.ascontiguousarray(x[b].T).astype(BF),
                "wqk": wqk_c.astype(BF),
                "bqk": np.ascontiguousarray(bqk_c, np.float32),
                "wv": wv_c.astype(BF),
                "bv": bv_c.astype(BF),
                "wp": np.ascontiguousarray(wp_c).astype(BF),
                "masks": masks.astype(BF),
            }
        )
    return in_maps


def kernel(x, Wqkv, bqkv, Wproj, bproj, _trace=False, _trace_out=None):
    from concourse.bass_utils import run_bass_kernel_spmd

    if "nc" not in _CACHE:
        _CACHE["nc"] = _build()
    nc = _CACHE["nc"]

    x = np.asarray(x, np.float32)
    Wqkv = np.asarray(Wqkv, np.float32)
    bqkv = np.asarray(bqkv, np.float32)
    Wproj = np.asarray(Wproj, np.float32)
    bproj = np.asarray(bproj, np.float32)

    in_maps = _prep_inputs(x, Wqkv, bqkv, Wproj)
    res = run_bass_kernel_spmd(
        nc, in_maps, core_ids=list(range(N_CORES)), trace=_trace
    )
    if _trace_out is not None:
        _trace_out.append(res)

    out = np.empty((B, T, D), np.float32)
    for b in range(B):
        acc = res.results[HPC * b]["out"].astype(np.float32)
        for g in range(1, HPC):
            acc = acc + res.results[HPC * b + g]["out"]
        out[b] = acc + bproj[None, :]
    return out


# revision 8
# speedup vs baseline: 1.4679x; 1.0478x over previous
"""Causal self-attention (B=2, T=2048, D=1024, H=16, hd=64) on 8 TRN2 cores.

Sharding: 2 batches x 4 head-groups (4 heads each). Each core computes the
full pipeline for its (batch, head-group); the host sums the 4 per-batch
partials (tensor-parallel reduce) and adds bproj.

v2 design (vs the f32r baseline):
 - bf16 everywhere on the PE (PSUM accumulation stays f32): q/k/v weights,
   activations, exp, proj. Validated ~3e-3 rel err vs the 2e-2 gate.
 - v is produced directly in natural [token, hd] layout (stationary = xT
   block, moving = Wv columns), with the +bv handled by a K=1 ones-row
   matmul. No PE transposes.
 - Attention runs head-pair-sequential per q-slice so only one y-accumulator
   psum tile is live at a time; its pool slot rotates per head-pair, which
   removes the slice-boundary stalls the baseline had.
 - Causal diagonal blocks slice the St/exp/y ranges instead of memsetting
   masked regions (bf16 matmuls run 1 cycle/row at any width).
 - Softmax normalization: DVE copies the two denominator rows out of PSUM,
   a K=1 bf16 matmul broadcasts them across 64 partitions, one
   reciprocal_approx_fast inverts both broadcasts, and two DVE muls write
   the normalized yT with the second head shifted to partitions 64:127 so
   the output projection contracts K=128 (two matmuls instead of four).
"""

import sys

sys.path.insert(0, "/opt/trn_rl_repo")

import numpy as np
import ml_dtypes
from collections import deque

B, T, D = 2, 2048, 1024
N_HEAD = 16
HD = 64  # head dim
HPC = 4  # heads per core
N_CORES = 8

P = 128
NJ = 512  # q-slice width
JT = T // NJ  # 4 q-slices
KT = D // P  # 8 contraction tiles
IT = T // P  # 16 token tiles

_CACHE = {}


def _build():
    import concourse.bass as bass  # noqa: F401
    import concourse.mybir as mybir
    import concourse.tile as tile
    from concourse import bacc

    F32 = mybir.dt.float32
    BF16 = mybir.dt.bfloat16
    AF = mybir.ActivationFunctionType

    nc = bacc.Bacc(None, target_bir_lowering=False)
    xT_d = nc.dram_tensor("xT", [D, T], BF16, kind="ExternalInput")
    wqk_d = nc.dram_tensor("wqk", [D, 4 * P], BF16, kind="ExternalInput")
    bqk_d = nc.dram_tensor("bqk", [P, 4], F32, kind="ExternalInput")
    wv_d = nc.dram_tensor("wv", [D, 4 * 65], BF16, kind="ExternalInput")
    bv_d = nc.dram_tensor("bv", [1, 4 * 65], BF16, kind="ExternalInput")
    wp_d = nc.dram_tensor("wp", [2 * P, D], BF16, kind="ExternalInput")
    masks_d = nc.dram_tensor("masks", [P, 2 * P], BF16, kind="ExternalInput")
    out_d = nc.dram_tensor("out", [T, D], F32, kind="ExternalOutput")

    with tile.TileContext(nc) as tc:
        with (
            tc.tile_pool(name="const", bufs=1) as const,
            tc.tile_pool(name="stp", bufs=2, space="PSUM") as stp,
            tc.tile_pool(name="yp", bufs=2, space="PSUM") as yp,
            tc.tile_pool(name="expp", bufs=6) as expp,
            tc.tile_pool(name="denp", bufs=2) as denp,
            tc.tile_pool(name="recp", bufs=2) as recp,
            tc.tile_pool(name="outp", bufs=2) as outp,
        ):
            w_sb = const.tile([P, KT, 4 * P], BF16)
            bqk_sb = const.tile([P, 4], F32)
            wv_sb = const.tile([P, KT, 4, 65], BF16)
            bv_sb = const.tile([P, 4, 65], BF16)
            wp_sb = const.tile([P, 2, D], BF16)
            masks_sb = const.tile([P, 2, P], BF16)
            ones_sb = const.tile([P, P], BF16)
            xt_sb = const.tile([P, KT, T], BF16)
            qkvT = const.tile([P, 4, T], BF16)
            vnat = const.tile([P, IT, 4, 65], BF16)
            yt2 = const.tile([P, 2, T], BF16)

            nc.gpsimd.memset(ones_sb[:], 1.0)
            nc.gpsimd.memset(vnat[:], 1.0)

            xT_r = xT_d.rearrange("(kt p) t -> p kt t", p=P)
            wqk_r = wqk_d.rearrange("(kt p) n -> p kt n", p=P)
            wv_r = wv_d.rearrange("(kt p) n -> p kt n", p=P)
            for k in range(KT):
                nc.sync.dma_start(xt_sb[:, k, 0:NJ], xT_r[:, k, 0:NJ])
                nc.sync.dma_start(w_sb[:, k, :], wqk_r[:, k, :])
                nc.sync.dma_start(wv_sb[:, k, :, :], wv_r[:, k, :])
            nc.sync.dma_start(bqk_sb[:], bqk_d[:])
            nc.sync.dma_start(bv_sb[0:1, :, :], bv_d[:])
            nc.sync.dma_start(
                wp_sb[:], wp_d.rearrange("(hp p) d -> p hp d", p=P)
            )
            nc.sync.dma_start(masks_sb[:], masks_d.rearrange("p (a b) -> p a b", a=2))

            def emit_xt(j):
                # One multi-dim DMA per slice: 8x fewer sync-queue issue slots.
                nc.sync.dma_start(
                    xt_sb[:, :, j * NJ : (j + 1) * NJ],
                    xT_r[:, :, j * NJ : (j + 1) * NJ],
                )

            # ---- fills: qkv q/k groups + natural-layout v tiles ------------
            def emit_qkv(j, m):
                ps = stp.tile([P, NJ], F32, tag="st", name=f"qkvps{j}_{m}")
                for k in range(KT):
                    nc.tensor.matmul(
                        ps[:],
                        w_sb[:, k, m * P : (m + 1) * P],
                        xt_sb[:, k, j * NJ : (j + 1) * NJ],
                        start=(k == 0),
                        stop=(k == KT - 1),
                    )
                with nc.allow_low_precision(reason="bf16 activations"):
                    nc.vector.tensor_scalar_add(
                        qkvT[:, m, j * NJ : (j + 1) * NJ], ps[:], bqk_sb[:, m : m + 1]
                    )

            def emit_v(ii):
                ps = stp.tile([P, 4, 65], F32, tag="st", name=f"vps{ii}")
                for k in range(KT):
                    nc.tensor.matmul(
                        ps[:],
                        xt_sb[:, k, ii * P : (ii + 1) * P],
                        wv_sb[:, k, :, :],
                        start=(k == 0),
                        stop=False,
                    )
                nc.tensor.matmul(
                    ps[:],
                    ones_sb[0:1, 0:P],
                    bv_sb[0:1, :, :],
                    start=False,
                    stop=True,
                )
                with nc.allow_low_precision(reason="bf16 activations"):
                    nc.vector.tensor_copy(vnat[:, ii, :, 0:HD], ps[:, :, 0:HD])

            fill_q = deque()

            def push_fill(j):
                fill_q.append(("qkv", j, 0))
                fill_q.append(("qkv", j, 2))
                for ii in range(4 * j, 4 * j + 4):
                    fill_q.append(("v", ii))
                fill_q.append(("qkv", j, 1))
                fill_q.append(("qkv", j, 3))

            def emit_fill(item):
                if item[0] == "qkv":
                    emit_qkv(item[1], item[2])
                else:
                    emit_v(item[1])

            proj_q = deque()

            def emit_proj(item):
                # po lives in the "st" ring: every st-slot tenant's releasing
                # reader is emitted in the same emit_* call, so a PE matmul
                # here can never wait on a not-yet-emitted instruction.
                qm, n = item
                po = stp.tile([P, NJ], F32, tag="st", name=f"po{qm}_{n}")
                for hp in range(2):
                    nc.tensor.matmul(
                        po[:],
                        yt2[:, hp, qm * P : (qm + 1) * P],
                        wp_sb[:, hp, n * NJ : (n + 1) * NJ],
                        start=(hp == 0),
                        stop=(hp == 1),
                    )
                ot = outp.tile([P, NJ], F32, tag="ot")
                nc.vector.tensor_copy(ot[:], po[:])
                nc.sync.dma_start(
                    out_d[qm * P : (qm + 1) * P, n * NJ : (n + 1) * NJ], ot[:]
                )

            def pump():
                if norm_q:
                    norm_q.popleft()()
                elif fill_q:
                    emit_fill(fill_q.popleft())
                elif proj_q:
                    emit_proj(proj_q.popleft())

            # ---- attention -------------------------------------------------
            def emit_st_exp(j, hp, i):
                r = i - 4 * j
                c0 = max(0, P * r)
                st = stp.tile([P, 2, NJ], F32, tag="st", name=f"st{j}_{hp}_{i}")
                for par in range(2):
                    rows = slice(HD * par, HD * par + HD)
                    nc.tensor.matmul(
                        st[:, par, c0:NJ],
                        qkvT[rows, 2 + hp, i * P : (i + 1) * P],
                        qkvT[rows, hp, j * NJ + c0 : (j + 1) * NJ],
                        start=True,
                        stop=True,
                        tile_position=(HD * par, 0),
                    )
                exp2 = expp.tile([P, 2, NJ], BF16, tag="exp")
                nc.scalar.activation(exp2[:, :, c0:NJ], st[:, :, c0:NJ], AF.Exp)
                if r >= 0:
                    nc.gpsimd.tensor_mul(
                        exp2[:, :, c0 : c0 + P],
                        exp2[:, :, c0 : c0 + P],
                        masks_sb[:],
                    )
                return exp2

            def emit_y(j, hp, i, exp2, y2, last):
                r = i - 4 * j
                c0 = max(0, P * r)
                for par in range(2):
                    nc.tensor.matmul(
                        y2[0:65, par, c0:NJ],
                        vnat[:, i, 2 * hp + par, :],
                        exp2[:, par, c0:NJ],
                        start=(i == 0),
                        stop=last,
                    )

            # Norm chain is emitted in stages pumped between later iterations
            # so its PE matmuls never block the St stream while the DVE den
            # copy / reciprocal latency drains.
            norm_q = deque()  # staged closures

            def emit_norm_stages(hp, j, y2):
                state = {}

                def s_den():
                    den = denp.tile([P, 2, NJ], BF16, tag="den")
                    with nc.allow_low_precision(reason="bf16 denominator"):
                        nc.vector.tensor_copy(den[HD:65, :, :], y2[HD:65, :, :])
                    state["den"] = den

                def s_bc():
                    bc = stp.tile([P, 2, NJ], F32, tag="st", name=f"bc{hp}_{j}")
                    den = state["den"]
                    for par in range(2):
                        nc.tensor.matmul(
                            bc[0:HD, par, :],
                            ones_sb[HD : HD + 1, 0:HD],
                            den[HD : HD + 1, par, :],
                            start=True,
                            stop=True,
                        )
                    state["bc"] = bc

                def s_rec():
                    rec = recp.tile([P, 2, NJ], F32, tag="rec")
                    nc.vector.reciprocal_approx_fast(
                        rec[0:HD, :, :], state["bc"][0:HD, :, :]
                    )
                    state["rec"] = rec

                def s_mul():
                    rec = state["rec"]
                    with nc.allow_low_precision(reason="bf16 yT"):
                        for par in range(2):
                            nc.vector.tensor_mul(
                                yt2[
                                    HD * par : HD * par + HD,
                                    hp,
                                    j * NJ : (j + 1) * NJ,
                                ],
                                y2[0:HD, par, :],
                                rec[0:HD, par, :],
                            )

                # den copy only waits on y2's stop; emit it immediately so the
                # chain starts draining, then pump the rest.
                s_den()
                norm_q.append(s_bc)
                norm_q.append(s_rec)
                norm_q.append(s_mul)

            DEPTH = 3
            y_q = deque()  # (hp, i, exp2, last)

            push_fill(0)
            while fill_q:
                emit_fill(fill_q.popleft())

            for j in range(JT):
                if j + 1 < JT:
                    emit_xt(j + 1)
                    push_fill(j + 1)
                n_i = 4 * j + 4
                for hp in range(2):
                    y2 = yp.tile([P, 2, NJ], F32, tag="y", name=f"y2_{hp}_{j}")
                    for i in range(n_i):
                        if len(y_q) > DEPTH:
                            rec_ = y_q.popleft()
                            emit_y(j, rec_[0], rec_[1], rec_[2], y2, rec_[3])
                        exp2 = emit_st_exp(j, hp, i)
                        y_q.append((hp, i, exp2, i == n_i - 1))
                        pump()
                    while y_q:
                        rec_ = y_q.popleft()
                        emit_y(j, rec_[0], rec_[1], rec_[2], y2, rec_[3])
                    emit_norm_stages(hp, j, y2)
                for qm in range(4 * j, 4 * j + 4):
                    proj_q.append((qm, 0))
                    proj_q.append((qm, 1))

            while norm_q:
                norm_q.popleft()()
            while fill_q:
                emit_fill(fill_q.popleft())
            while proj_q:
                emit_proj(proj_q.popleft())

    nc.compile()
    return nc


def _prep_inputs(x, Wqkv, bqkv, Wproj):
    """Per-core input maps. Core c -> batch c//4, heads 4*(c%4) .. +4."""
    BF = ml_dtypes.bfloat16
    scale = np.float32(1.0 / np.sqrt(HD))
    pp = np.arange(P)[:, None]
    ff = np.arange(P)[None, :]
    tri = (ff >= pp).astype(np.float32)
    masks = np.concatenate([tri, tri], axis=1)

    in_maps = []
    for c in range(N_CORES):
        b, g = divmod(c, HPC)
        cs = slice(256 * g, 256 * g + 256)
        wq = Wqkv[:, 0 * D :][:, cs] * scale
        wk = Wqkv[:, 1 * D : 2 * D][:, cs]
        wv = Wqkv[:, 2 * D : 3 * D][:, cs]
        wqk_c = np.ascontiguousarray(np.concatenate([wq, wk], axis=1))
        bq = bqkv[0 * D :][cs] * scale
        bk = bqkv[1 * D : 2 * D][cs]
        bv = bqkv[2 * D : 3 * D][cs]
        bqk_c = np.concatenate([bq, bk]).reshape(4, P).T
        wv_c = np.zeros((D, 4 * 65), np.float32)
        bv_c = np.zeros((1, 4 * 65), np.float32)
        for h in range(4):
            wv_c[:, 65 * h : 65 * h + HD] = wv[:, HD * h : HD * (h + 1)]
            bv_c[0, 65 * h : 65 * h + HD] = bv[HD * h : HD * (h + 1)]
        wp_c = Wproj[256 * g : 256 * (g + 1), :]
        in_maps.append(
            {
                "xT": np.ascontiguousarray(x[b].T).astype(BF),
                "wqk": wqk_c.astype(BF),
                "bqk": np.ascontiguousarray(bqk_c, np.float32),
                "wv": wv_c.astype(BF),
                "bv": bv_c.astype(BF),
                "wp": np.ascontiguousarray(wp_c).astype(BF),
                "masks": masks.astype(BF),
            }
        )
    return in_maps


def kernel(x, Wqkv, bqkv, Wproj, bproj, _trace=False, _trace_out=None):
    from concourse.bass_utils import run_bass_kernel_spmd

    if "nc" not in _CACHE:
        _CACHE["nc"] = _build()
    nc = _CACHE["nc"]

    x = np.asarray(x, np.float32)
    Wqkv = np.asarray(Wqkv, np.float32)
    bqkv = np.asarray(bqkv, np.float32)
    Wproj = np.asarray(Wproj, np.float32)
    bproj = np.asarray(bproj, np.float32)

    in_maps = _prep_inputs(x, Wqkv, bqkv, Wproj)
    res = run_bass_kernel_spmd(
        nc, in_maps, core_ids=list(range(N_CORES)), trace=_trace
    )
    if _trace_out is not None:
        _trace_out.append(res)

    out = np.empty((B, T, D), np.float32)
    for b in range(B):
        acc = res.results[HPC * b]["out"].astype(np.float32)
        for g in range(1, HPC):
            acc = acc + res.results[HPC * b + g]["out"]
        out[b] = acc + bproj[None, :]
    return out


# revision 16
# speedup vs baseline: 1.5516x; 1.0570x over previous
"""Causal self-attention (B=2, T=2048, D=1024, H=16, hd=64) on 8 TRN2 cores.

Sharding: 2 batches x 4 head-groups (4 heads each). Each core computes the
full pipeline for its (batch, head-group); the host sums the 4 per-batch
partials (tensor-parallel reduce) and adds bproj.

v2 design (vs the f32r baseline):
 - bf16 everywhere on the PE (PSUM accumulation stays f32): q/k/v weights,
   activations, exp, proj. Validated ~3e-3 rel err vs the 2e-2 gate.
 - v is produced directly in natural [token, hd] layout (stationary = xT
   block, moving = Wv columns), with the +bv handled by a K=1 ones-row
   matmul. No PE transposes.
 - Attention runs head-pair-sequential per q-slice so only one y-accumulator
   psum tile is live at a time; its pool slot rotates per head-pair, which
   removes the slice-boundary stalls the baseline had.
 - Causal diagonal blocks slice the St/exp/y ranges instead of memsetting
   masked regions (bf16 matmuls run 1 cycle/row at any width).
 - Softmax normalization: DVE copies the two denominator rows out of PSUM,
   a K=1 bf16 matmul broadcasts them across 64 partitions, one
   reciprocal_approx_fast inverts both broadcasts, and two DVE muls write
   the normalized yT with the second head shifted to partitions 64:127 so
   the output projection contracts K=128 (two matmuls instead of four).
"""

import sys

sys.path.insert(0, "/opt/trn_rl_repo")

import numpy as np
import ml_dtypes
from collections import deque

B, T, D = 2, 2048, 1024
N_HEAD = 16
HD = 64  # head dim
HPC = 4  # heads per core
N_CORES = 8

P = 128
NJ = 512  # q-slice width
JT = T // NJ  # 4 q-slices
KT = D // P  # 8 contraction tiles
IT = T // P  # 16 token tiles

_CACHE = {}


def _build():
    import concourse.bass as bass  # noqa: F401
    import concourse.mybir as mybir
    import concourse.tile as tile
    from concourse import bacc

    F32 = mybir.dt.float32
    BF16 = mybir.dt.bfloat16
    AF = mybir.ActivationFunctionType

    nc = bacc.Bacc(None, target_bir_lowering=False)
    xT_d = nc.dram_tensor("xT", [D, T], BF16, kind="ExternalInput")
    wqk_d = nc.dram_tensor("wqk", [D, 4 * P], BF16, kind="ExternalInput")
    bqk_d = nc.dram_tensor("bqk", [P, 4], F32, kind="ExternalInput")
    wv_d = nc.dram_tensor("wv", [D, 4 * 65], BF16, kind="ExternalInput")
    bv_d = nc.dram_tensor("bv", [1, 4 * 65], BF16, kind="ExternalInput")
    wp_d = nc.dram_tensor("wp", [2 * P, D], BF16, kind="ExternalInput")
    masks_d = nc.dram_tensor("masks", [P, 2 * P], BF16, kind="ExternalInput")
    out_d = nc.dram_tensor("out", [T, D], F32, kind="ExternalOutput")

    with tile.TileContext(nc) as tc:
        with (
            tc.tile_pool(name="const", bufs=1) as const,
            tc.tile_pool(name="stp", bufs=2, space="PSUM") as stp,
            tc.tile_pool(name="yp", bufs=2, space="PSUM") as yp,
            tc.tile_pool(name="expp", bufs=6) as expp,
            tc.tile_pool(name="denp", bufs=2) as denp,
            tc.tile_pool(name="recp", bufs=2) as recp,
            tc.tile_pool(name="outp", bufs=2) as outp,
        ):
            w_sb = const.tile([P, KT, 4 * P], BF16)
            bqk_sb = const.tile([P, 4], F32)
            wv_sb = const.tile([P, KT, 4, 65], BF16)
            bv_sb = const.tile([P, 4, 65], BF16)
            wp_sb = const.tile([P, 2, D], BF16)
            masks_sb = const.tile([P, 2, P], BF16)
            ones_sb = const.tile([P, P], BF16)
            xt_sb = const.tile([P, KT, T], BF16)
            qkvT = const.tile([P, 4, T], BF16)
            vnat = const.tile([P, IT, 4, 65], BF16)
            yt2 = const.tile([P, 2, T], BF16)

            nc.gpsimd.memset(ones_sb[:], 1.0)
            nc.gpsimd.memset(vnat[:], 1.0)

            xT_r = xT_d.rearrange("(kt p) t -> p kt t", p=P)
            wqk_r = wqk_d.rearrange("(kt p) n -> p kt n", p=P)
            wv_r = wv_d.rearrange("(kt p) n -> p kt n", p=P)
            # Merged multi-dim DMAs: the sync queue issues each PSEUDO_DMA in
            # ~600ns, so per-k transfers serialize for ~16us; merged ones
            # issue once and let the DMA engine stream.
            nc.sync.dma_start(w_sb[:, 0:2, :], wqk_r[:, 0:2, :])
            nc.sync.dma_start(xt_sb[:, 0:2, 0:NJ], xT_r[:, 0:2, 0:NJ])
            nc.sync.dma_start(w_sb[:, 2:KT, :], wqk_r[:, 2:KT, :])
            nc.sync.dma_start(xt_sb[:, 2:KT, 0:NJ], xT_r[:, 2:KT, 0:NJ])
            nc.sync.dma_start(wv_sb[:, :, :, :], wv_r[:, :, :])
            nc.sync.dma_start(bqk_sb[:], bqk_d[:])
            nc.sync.dma_start(bv_sb[0:1, :, :], bv_d[:])
            nc.sync.dma_start(
                wp_sb[:], wp_d.rearrange("(hp p) d -> p hp d", p=P)
            )
            nc.sync.dma_start(masks_sb[:], masks_d.rearrange("p (a b) -> p a b", a=2))

            def emit_xt(j):
                # One multi-dim DMA per slice: 8x fewer sync-queue issue slots.
                nc.sync.dma_start(
                    xt_sb[:, :, j * NJ : (j + 1) * NJ],
                    xT_r[:, :, j * NJ : (j + 1) * NJ],
                )

            # ---- fills: qkv q/k groups + natural-layout v tiles ------------
            def emit_qkv(j, m):
                ps = stp.tile([P, NJ], F32, tag="st", name=f"qkvps{j}_{m}")
                for k in range(KT):
                    nc.tensor.matmul(
                        ps[:],
                        w_sb[:, k, m * P : (m + 1) * P],
                        xt_sb[:, k, j * NJ : (j + 1) * NJ],
                        start=(k == 0),
                        stop=(k == KT - 1),
                    )
                with nc.allow_low_precision(reason="bf16 activations"):
                    nc.vector.tensor_scalar_add(
                        qkvT[:, m, j * NJ : (j + 1) * NJ], ps[:], bqk_sb[:, m : m + 1]
                    )

            def emit_v(ii):
                ps = stp.tile([P, 4, 65], F32, tag="st", name=f"vps{ii}")
                for k in range(KT):
                    nc.tensor.matmul(
                        ps[:],
                        xt_sb[:, k, ii * P : (ii + 1) * P],
                        wv_sb[:, k, :, :],
                        start=(k == 0),
                        stop=False,
                    )
                nc.tensor.matmul(
                    ps[:],
                    ones_sb[0:1, 0:P],
                    bv_sb[0:1, :, :],
                    start=False,
                    stop=True,
                )
                with nc.allow_low_precision(reason="bf16 activations"):
                    nc.vector.tensor_copy(vnat[:, ii, :, 0:HD], ps[:, :, 0:HD])

            fill_q = deque()

            def push_fill(j):
                fill_q.append(("qkv", j, 0))
                fill_q.append(("qkv", j, 2))
                for ii in range(4 * j, 4 * j + 4):
                    fill_q.append(("v", ii))
                fill_q.append(("qkv", j, 1))
                fill_q.append(("qkv", j, 3))

            def emit_fill(item):
                if item[0] == "qkv":
                    emit_qkv(item[1], item[2])
                else:
                    emit_v(item[1])

            proj_q = deque()

            def emit_proj(item):
                # po lives in the "st" ring: every st-slot tenant's releasing
                # reader is emitted in the same emit_* call, so a PE matmul
                # here can never wait on a not-yet-emitted instruction.
                qm, n = item
                po = stp.tile([P, NJ], F32, tag="st", name=f"po{qm}_{n}")
                for hp in range(2):
                    nc.tensor.matmul(
                        po[:],
                        yt2[:, hp, qm * P : (qm + 1) * P],
                        wp_sb[:, hp, n * NJ : (n + 1) * NJ],
                        start=(hp == 0),
                        stop=(hp == 1),
                    )
                ot = outp.tile([P, NJ], F32, tag="ot")
                nc.vector.tensor_copy(ot[:], po[:])
                nc.sync.dma_start(
                    out_d[qm * P : (qm + 1) * P, n * NJ : (n + 1) * NJ], ot[:]
                )

            def pump():
                tick[0] += 1
                if norm_q and norm_q[0][0] <= tick[0]:
                    norm_q.popleft()[1]()
                if fill_q:
                    emit_fill(fill_q.popleft())
                elif proj_q:
                    emit_proj(proj_q.popleft())

            # ---- attention -------------------------------------------------
            def emit_st_exp(j, hp, i):
                r = i - 4 * j
                c0 = max(0, P * r)
                st = stp.tile([P, 2, NJ], F32, tag="st", name=f"st{j}_{hp}_{i}")
                for par in range(2):
                    rows = slice(HD * par, HD * par + HD)
                    nc.tensor.matmul(
                        st[:, par, c0:NJ],
                        qkvT[rows, 2 + hp, i * P : (i + 1) * P],
                        qkvT[rows, hp, j * NJ + c0 : (j + 1) * NJ],
                        start=True,
                        stop=True,
                        tile_position=(HD * par, 0),
                    )
                exp2 = expp.tile([P, 2, NJ], BF16, tag="exp")
                nc.scalar.activation(exp2[:, :, c0:NJ], st[:, :, c0:NJ], AF.Exp)
                if r >= 0:
                    nc.gpsimd.tensor_mul(
                        exp2[:, :, c0 : c0 + P],
                        exp2[:, :, c0 : c0 + P],
                        masks_sb[:],
                    )
                return exp2

            def emit_y(j, hp, i, exp2, y2, last):
                r = i - 4 * j
                c0 = max(0, P * r)
                for par in range(2):
                    nc.tensor.matmul(
                        y2[0:65, par, c0:NJ],
                        vnat[:, i, 2 * hp + par, :],
                        exp2[:, par, c0:NJ],
                        start=(i == 0),
                        stop=last,
                    )

            # Norm chain is emitted in stages pumped between later iterations
            # so its PE matmuls never block the St stream while the DVE den
            # copy / reciprocal latency drains. Each stage carries a due tick:
            # the PE queue is in-order, so a bc matmul emitted too early still
            # stalls the engine behind the 1.2us den copy.
            norm_q = deque()  # (due_tick, closure)
            tick = [0]

            def emit_norm_stages(hp, j, y2):
                # At most one head-pair's stages may be pending: drain the
                # previous ones so y-slot release ops are always emitted
                # before the slot's next-next tenant allocates.
                while norm_q:
                    norm_q.popleft()[1]()
                state = {}

                def s_den():
                    den = denp.tile([P, 2, NJ], BF16, tag="den")
                    with nc.allow_low_precision(reason="bf16 denominator"):
                        nc.vector.tensor_copy(den[HD:65, :, :], y2[HD:65, :, :])
                    state["den"] = den

                def s_bc():
                    bc = stp.tile([P, 2, NJ], F32, tag="st", name=f"bc{hp}_{j}")
                    den = state["den"]
                    for par in range(2):
                        nc.tensor.matmul(
                            bc[0:HD, par, :],
                            ones_sb[HD : HD + 1, 0:HD],
                            den[HD : HD + 1, par, :],
                            start=True,
                            stop=True,
                        )
                    state["bc"] = bc

                def s_rec():
                    rec = recp.tile([P, 2, NJ], F32, tag="rec")
                    nc.vector.reciprocal_approx_fast(
                        rec[0:HD, :, :], state["bc"][0:HD, :, :]
                    )
                    state["rec"] = rec

                def s_mul():
                    rec = state["rec"]
                    with nc.allow_low_precision(reason="bf16 yT"):
                        for par in range(2):
                            nc.vector.tensor_mul(
                                yt2[
                                    HD * par : HD * par + HD,
                                    hp,
                                    j * NJ : (j + 1) * NJ,
                                ],
                                y2[0:HD, par, :],
                                rec[0:HD, par, :],
                            )
                    if hp == 1:
                        # Projections read yt2 slice j; queue them only once
                        # both head-pairs' norm muls are emitted (Tile deps
                        # follow emission order).
                        for qm in range(4 * j, 4 * j + 4):
                            proj_q.append((qm, 0))
                            proj_q.append((qm, 1))

                # den copy only waits on y2's stop; emit it immediately so the
                # chain starts draining, then pump the rest with spacing.
                s_den()
                t = tick[0]
                norm_q.append((t + 2, s_bc))
                norm_q.append((t + 4, s_rec))
                norm_q.append((t + 5, s_mul))

            DEPTH = 3
            y_q = deque()  # (hp, i, exp2, last)

            push_fill(0)
            while fill_q:
                emit_fill(fill_q.popleft())

            for j in range(JT):
                if j + 1 < JT:
                    emit_xt(j + 1)
                    push_fill(j + 1)
                n_i = 4 * j + 4
                for hp in range(2):
                    y2 = yp.tile([P, 2, NJ], F32, tag="y", name=f"y2_{hp}_{j}")
                    for i in range(n_i):
                        if len(y_q) > DEPTH:
                            rec_ = y_q.popleft()
                            emit_y(j, rec_[0], rec_[1], rec_[2], y2, rec_[3])
                        exp2 = emit_st_exp(j, hp, i)
                        y_q.append((hp, i, exp2, i == n_i - 1))
                        pump()
                    while y_q:
                        rec_ = y_q.popleft()
                        emit_y(j, rec_[0], rec_[1], rec_[2], y2, rec_[3])
                    emit_norm_stages(hp, j, y2)

            while norm_q:
                norm_q.popleft()[1]()
            while fill_q:
                emit_fill(fill_q.popleft())
            while proj_q:
                emit_proj(proj_q.popleft())

    nc.compile()
    return nc


def _prep_inputs(x, Wqkv, bqkv, Wproj):
    """Per-core input maps. Core c -> batch c//4, heads 4*(c%4) .. +4."""
    BF = ml_dtypes.bfloat16
    scale = np.float32(1.0 / np.sqrt(HD))
    pp = np.arange(P)[:, None]
    ff = np.arange(P)[None, :]
    tri = (ff >= pp).astype(np.float32)
    masks = np.concatenate([tri, tri], axis=1)

    in_maps = []
    for c in range(N_CORES):
        b, g = divmod(c, HPC)
        cs = slice(256 * g, 256 * g + 256)
        wq = Wqkv[:, 0 * D :][:, cs] * scale
        wk = Wqkv[:, 1 * D : 2 * D][:, cs]
        wv = Wqkv[:, 2 * D : 3 * D][:, cs]
        wqk_c = np.ascontiguousarray(np.concatenate([wq, wk], axis=1))
        bq = bqkv[0 * D :][cs] * scale
        bk = bqkv[1 * D : 2 * D][cs]
        bv = bqkv[2 * D : 3 * D][cs]
        bqk_c = np.concatenate([bq, bk]).reshape(4, P).T
        wv_c = np.zeros((D, 4 * 65), np.float32)
        bv_c = np.zeros((1, 4 * 65), np.float32)
        for h in range(4):
            wv_c[:, 65 * h : 65 * h + HD] = wv[:, HD * h : HD * (h + 1)]
            bv_c[0, 65 * h : 65 * h + HD] = bv[HD * h : HD * (h + 1)]
        wp_c = Wproj[256 * g : 256 * (g + 1), :]
        in_maps.append(
            {
                "xT": np.ascontiguousarray(x[b].T).astype(BF),
                "wqk": wqk_c.astype(BF),
                "bqk": np.ascontiguousarray(bqk_c, np.float32),
                "wv": wv_c.astype(BF),
                "bv": bv_c.astype(BF),
                "wp": np.ascontiguousarray(wp_c).astype(BF),
                "masks": masks.astype(BF),
            }
        )
    return in_maps


def kernel(x, Wqkv, bqkv, Wproj, bproj, _trace=False, _trace_out=None):
    from concourse.bass_utils import run_bass_kernel_spmd

    if "nc" not in _CACHE:
        _CACHE["nc"] = _build()
    nc = _CACHE["nc"]

    x = np.asarray(x, np.float32)
    Wqkv = np.asarray(Wqkv, np.float32)
    bqkv = np.asarray(bqkv, np.float32)
    Wproj = np.asarray(Wproj, np.float32)
    bproj = np.asarray(bproj, np.float32)

    in_maps = _prep_inputs(x, Wqkv, bqkv, Wproj)
    res = run_bass_kernel_spmd(
        nc, in_maps, core_ids=list(range(N_CORES)), trace=_trace
    )
    if _trace_out is not None:
        _trace_out.append(res)

    out = np.empty((B, T, D), np.float32)
    for b in range(B):
        acc = res.results[HPC * b]["out"].astype(np.float32)
        for g in range(1, HPC):
            acc = acc + res.results[HPC * b + g]["out"]
        out[b] = acc + bproj[None, :]
    return out


# revision 21
# speedup vs baseline: 1.6146x; 1.0406x over previous
"""Causal self-attention (B=2, T=2048, D=1024, H=16, hd=64) on 8 TRN2 cores.

Sharding: 2 batches x 4 head-groups (4 heads each). Each core computes the
full pipeline for its (batch, head-group); the host sums the 4 per-batch
partials (tensor-parallel reduce) and adds bproj.

v2 design (vs the f32r baseline):
 - bf16 everywhere on the PE (PSUM accumulation stays f32): q/k/v weights,
   activations, exp, proj. Validated ~3e-3 rel err vs the 2e-2 gate.
 - v is produced directly in natural [token, hd] layout (stationary = xT
   block, moving = Wv columns), with the +bv handled by a K=1 ones-row
   matmul. No PE transposes.
 - Attention runs head-pair-sequential per q-slice so only one y-accumulator
   psum tile is live at a time; its pool slot rotates per head-pair, which
   removes the slice-boundary stalls the baseline had.
 - Causal diagonal blocks slice the St/exp/y ranges instead of memsetting
   masked regions (bf16 matmuls run 1 cycle/row at any width).
 - Softmax normalization: DVE copies the two denominator rows out of PSUM,
   a K=1 bf16 matmul broadcasts them across 64 partitions, one
   reciprocal_approx_fast inverts both broadcasts, and two DVE muls write
   the normalized yT with the second head shifted to partitions 64:127 so
   the output projection contracts K=128 (two matmuls instead of four).
"""

import sys

sys.path.insert(0, "/opt/trn_rl_repo")

import numpy as np
import ml_dtypes
from collections import deque

B, T, D = 2, 2048, 1024
N_HEAD = 16
HD = 64  # head dim
HPC = 4  # heads per core
N_CORES = 8

P = 128
NJ = 512  # q-slice width
JT = T // NJ  # 4 q-slices
KT = D // P  # 8 contraction tiles
IT = T // P  # 16 token tiles

_CACHE = {}


def _build():
    import concourse.bass as bass  # noqa: F401
    import concourse.mybir as mybir
    import concourse.tile as tile
    from concourse import bacc

    F32 = mybir.dt.float32
    BF16 = mybir.dt.bfloat16
    AF = mybir.ActivationFunctionType

    nc = bacc.Bacc(None, target_bir_lowering=False)
    xT_d = nc.dram_tensor("xT", [D, T], BF16, kind="ExternalInput")
    wqk_d = nc.dram_tensor("wqk", [D, 4 * P], BF16, kind="ExternalInput")
    bqk_d = nc.dram_tensor("bqk", [P, 4], F32, kind="ExternalInput")
    wv_d = nc.dram_tensor("wv", [D, 4 * 65], BF16, kind="ExternalInput")
    bv_d = nc.dram_tensor("bv", [1, 4 * 65], BF16, kind="ExternalInput")
    wp_d = nc.dram_tensor("wp", [2 * P, D], BF16, kind="ExternalInput")
    masks_d = nc.dram_tensor("masks", [P, 2 * P], BF16, kind="ExternalInput")
    out_d = nc.dram_tensor("out", [T, D], F32, kind="ExternalOutput")

    with tile.TileContext(nc) as tc:
        with (
            tc.tile_pool(name="const", bufs=1) as const,
            tc.tile_pool(name="stp", bufs=2, space="PSUM") as stp,
            tc.tile_pool(name="yp", bufs=2, space="PSUM") as yp,
            tc.tile_pool(name="expp", bufs=7) as expp,
            tc.tile_pool(name="denp", bufs=2) as denp,
            tc.tile_pool(name="recp", bufs=2) as recp,
            tc.tile_pool(name="outp", bufs=2) as outp,
        ):
            w_sb = const.tile([P, KT, 4 * P], BF16)
            bqk_sb = const.tile([P, 4], F32)
            wv_sb = const.tile([P, KT, 4, 65], BF16)
            bv_sb = const.tile([P, 4, 65], BF16)
            wp_sb = const.tile([P, 2, D], BF16)
            masks_sb = const.tile([P, 2, P], BF16)
            ones_sb = const.tile([P, P], BF16)
            xt_sb = const.tile([P, KT, T], BF16)
            qkvT = const.tile([P, 4, T], BF16)
            vnat = const.tile([P, IT, 4, 65], BF16)
            yt2 = const.tile([P, 2, T], BF16)

            nc.gpsimd.memset(ones_sb[:], 1.0)
            nc.gpsimd.memset(vnat[:], 1.0)

            xT_r = xT_d.rearrange("(kt p) t -> p kt t", p=P)
            wqk_r = wqk_d.rearrange("(kt p) n -> p kt n", p=P)
            wv_r = wv_d.rearrange("(kt p) n -> p kt n", p=P)
            # Merged multi-dim DMAs: the sync queue issues each PSEUDO_DMA in
            # ~600ns, so per-k transfers serialize for ~16us; merged ones
            # issue once and let the DMA engine stream.
            nc.sync.dma_start(bqk_sb[:], bqk_d[:])
            for k0, k1 in ((0, 2), (2, 5), (5, 8)):
                nc.sync.dma_start(w_sb[:, k0:k1, :], wqk_r[:, k0:k1, :])
                nc.sync.dma_start(xt_sb[:, k0:k1, 0:NJ], xT_r[:, k0:k1, 0:NJ])
            nc.sync.dma_start(wv_sb[:, :, :, :], wv_r[:, :, :])
            nc.sync.dma_start(bv_sb[0:1, :, :], bv_d[:])
            nc.sync.dma_start(
                wp_sb[:], wp_d.rearrange("(hp p) d -> p hp d", p=P)
            )
            nc.sync.dma_start(masks_sb[:], masks_d.rearrange("p (a b) -> p a b", a=2))

            def emit_xt(j):
                # One multi-dim DMA per slice: 8x fewer sync-queue issue slots.
                nc.sync.dma_start(
                    xt_sb[:, :, j * NJ : (j + 1) * NJ],
                    xT_r[:, :, j * NJ : (j + 1) * NJ],
                )

            # ---- fills: qkv q/k groups + natural-layout v tiles ------------
            def emit_qkv(j, m):
                ps = stp.tile([P, NJ], F32, tag="st", name=f"qkvps{j}_{m}")
                for k in range(KT):
                    nc.tensor.matmul(
                        ps[:],
                        w_sb[:, k, m * P : (m + 1) * P],
                        xt_sb[:, k, j * NJ : (j + 1) * NJ],
                        start=(k == 0),
                        stop=(k == KT - 1),
                    )
                with nc.allow_low_precision(reason="bf16 activations"):
                    nc.vector.tensor_scalar_add(
                        qkvT[:, m, j * NJ : (j + 1) * NJ], ps[:], bqk_sb[:, m : m + 1]
                    )

            def emit_v(ii):
                ps = stp.tile([P, 4, 65], F32, tag="st", name=f"vps{ii}")
                for k in range(KT):
                    nc.tensor.matmul(
                        ps[:],
                        xt_sb[:, k, ii * P : (ii + 1) * P],
                        wv_sb[:, k, :, :],
                        start=(k == 0),
                        stop=False,
                    )
                nc.tensor.matmul(
                    ps[:],
                    ones_sb[0:1, 0:P],
                    bv_sb[0:1, :, :],
                    start=False,
                    stop=True,
                )
                with nc.allow_low_precision(reason="bf16 activations"):
                    nc.vector.tensor_copy(vnat[:, ii, :, 0:HD], ps[:, :, 0:HD])

            fill_q = deque()

            def push_fill(j):
                fill_q.append(("qkv", j, 0))
                fill_q.append(("qkv", j, 2))
                for ii in range(4 * j, 4 * j + 4):
                    fill_q.append(("v", ii))
                fill_q.append(("qkv", j, 1))
                fill_q.append(("qkv", j, 3))

            def emit_fill(item):
                if item[0] == "qkv":
                    emit_qkv(item[1], item[2])
                else:
                    emit_v(item[1])

            proj_q = deque()

            def emit_proj(item):
                # po lives in the "st" ring: every st-slot tenant's releasing
                # reader is emitted in the same emit_* call, so a PE matmul
                # here can never wait on a not-yet-emitted instruction.
                qm, n = item
                po = stp.tile([P, NJ], F32, tag="st", name=f"po{qm}_{n}")
                for hp in range(2):
                    nc.tensor.matmul(
                        po[:],
                        yt2[:, hp, qm * P : (qm + 1) * P],
                        wp_sb[:, hp, n * NJ : (n + 1) * NJ],
                        start=(hp == 0),
                        stop=(hp == 1),
                    )
                ot = outp.tile([P, NJ], F32, tag="ot")
                nc.vector.tensor_copy(ot[:], po[:])
                nc.sync.dma_start(
                    out_d[qm * P : (qm + 1) * P, n * NJ : (n + 1) * NJ], ot[:]
                )

            def pump():
                tick[0] += 1
                if norm_q and norm_q[0][0] <= tick[0]:
                    norm_q.popleft()[1]()
                if fill_q:
                    emit_fill(fill_q.popleft())
                elif proj_q:
                    emit_proj(proj_q.popleft())

            # ---- attention -------------------------------------------------
            def emit_st_exp(j, hp, i):
                r = i - 4 * j
                c0 = max(0, P * r)
                st = stp.tile([P, 2, NJ], F32, tag="st", name=f"st{j}_{hp}_{i}")
                for par in range(2):
                    rows = slice(HD * par, HD * par + HD)
                    nc.tensor.matmul(
                        st[:, par, c0:NJ],
                        qkvT[rows, 2 + hp, i * P : (i + 1) * P],
                        qkvT[rows, hp, j * NJ + c0 : (j + 1) * NJ],
                        start=True,
                        stop=True,
                        tile_position=(HD * par, 0),
                    )
                exp2 = expp.tile([P, 2, NJ], BF16, tag="exp")
                nc.scalar.activation(exp2[:, :, c0:NJ], st[:, :, c0:NJ], AF.Exp)
                if r >= 0:
                    nc.gpsimd.tensor_mul(
                        exp2[:, :, c0 : c0 + P],
                        exp2[:, :, c0 : c0 + P],
                        masks_sb[:],
                    )
                return exp2

            def emit_y(j, hp, i, exp2, y2, last):
                r = i - 4 * j
                c0 = max(0, P * r)
                for par in range(2):
                    nc.tensor.matmul(
                        y2[0:65, par, c0:NJ],
                        vnat[:, i, 2 * hp + par, :],
                        exp2[:, par, c0:NJ],
                        start=(i == 0),
                        stop=last,
                    )

            def pop_y():
                j_, hp_, i_, exp2_, y2_, last_ = y_q.popleft()
                emit_y(j_, hp_, i_, exp2_, y2_, last_)
                if last_:
                    emit_norm_stages(hp_, j_, y2_)

            # Norm chain is emitted in stages pumped between later iterations
            # so its PE matmuls never block the St stream while the DVE den
            # copy / reciprocal latency drains. Each stage carries a due tick:
            # the PE queue is in-order, so a bc matmul emitted too early still
            # stalls the engine behind the 1.2us den copy.
            norm_q = deque()  # (due_tick, closure)
            tick = [0]

            def emit_norm_stages(hp, j, y2):
                # At most one head-pair's stages may be pending: drain the
                # previous ones so y-slot release ops are always emitted
                # before the slot's next-next tenant allocates.
                while norm_q:
                    norm_q.popleft()[1]()
                state = {}

                def s_den():
                    den = denp.tile([P, 2, NJ], BF16, tag="den")
                    with nc.allow_low_precision(reason="bf16 denominator"):
                        nc.vector.tensor_copy(den[HD:65, :, :], y2[HD:65, :, :])
                    state["den"] = den

                def s_bc():
                    bc = stp.tile([P, 2, NJ], F32, tag="st", name=f"bc{hp}_{j}")
                    den = state["den"]
                    for par in range(2):
                        nc.tensor.matmul(
                            bc[0:HD, par, :],
                            ones_sb[HD : HD + 1, 0:HD],
                            den[HD : HD + 1, par, :],
                            start=True,
                            stop=True,
                        )
                    state["bc"] = bc

                def s_rec():
                    rec = recp.tile([P, 2, NJ], F32, tag="rec")
                    nc.vector.reciprocal_approx_fast(
                        rec[0:HD, :, :], state["bc"][0:HD, :, :]
                    )
                    state["rec"] = rec

                def s_mul():
                    rec = state["rec"]
                    with nc.allow_low_precision(reason="bf16 yT"):
                        for par in range(2):
                            nc.vector.tensor_mul(
                                yt2[
                                    HD * par : HD * par + HD,
                                    hp,
                                    j * NJ : (j + 1) * NJ,
                                ],
                                y2[0:HD, par, :],
                                rec[0:HD, par, :],
                            )
                    if hp == 1:
                        # Projections read yt2 slice j; queue them only once
                        # both head-pairs' norm muls are emitted (Tile deps
                        # follow emission order).
                        for qm in range(4 * j, 4 * j + 4):
                            proj_q.append((qm, 0))
                            proj_q.append((qm, 1))

                # den copy only waits on y2's stop; emit it immediately so the
                # chain starts draining, then pump the rest with spacing.
                s_den()
                t = tick[0]
                norm_q.append((t + 3, s_bc))
                norm_q.append((t + 5, s_rec))
                norm_q.append((t + 6, s_mul))

            DEPTH = 4
            y_q = deque()  # (j, hp, i, exp2, y2, last) — trails across hp

            push_fill(0)
            while fill_q:
                emit_fill(fill_q.popleft())

            for j in range(JT):
                if j + 1 < JT:
                    emit_xt(j + 1)
                    push_fill(j + 1)
                n_i = 4 * j + 4
                for hp in range(2):
                    # At hp start norm_q can only hold stages from two
                    # head-pairs ago (the previous hp's aren't pushed yet):
                    # drain them so this hp's y2 slot releases are emitted
                    # before its first write.
                    while norm_q:
                        norm_q.popleft()[1]()
                    y2 = yp.tile([P, 2, NJ], F32, tag="y", name=f"y2_{hp}_{j}")
                    for i in range(n_i):
                        if len(y_q) > DEPTH:
                            pop_y()
                        exp2 = emit_st_exp(j, hp, i)
                        y_q.append((j, hp, i, exp2, y2, i == n_i - 1))
                        pump()

            while y_q:
                pop_y()
            while norm_q:
                norm_q.popleft()[1]()
            while fill_q:
                emit_fill(fill_q.popleft())
            while proj_q:
                emit_proj(proj_q.popleft())

    nc.compile()
    return nc


def _prep_inputs(x, Wqkv, bqkv, Wproj):
    """Per-core input maps. Core c -> batch c//4, heads 4*(c%4) .. +4."""
    BF = ml_dtypes.bfloat16
    scale = np.float32(1.0 / np.sqrt(HD))
    pp = np.arange(P)[:, None]
    ff = np.arange(P)[None, :]
    tri = (ff >= pp).astype(np.float32)
    masks = np.concatenate([tri, tri], axis=1)

    in_maps = []
    for c in range(N_CORES):
        b, g = divmod(c, HPC)
        cs = slice(256 * g, 256 * g + 256)
        wq = Wqkv[:, 0 * D :][:, cs] * scale
        wk = Wqkv[:, 1 * D : 2 * D][:, cs]
        wv = Wqkv[:, 2 * D : 3 * D][:, cs]
        wqk_c = np.ascontiguousarray(np.concatenate([wq, wk], axis=1))
        bq = bqkv[0 * D :][cs] * scale
        bk = bqkv[1 * D : 2 * D][cs]
        bv = bqkv[2 * D : 3 * D][cs]
        bqk_c = np.concatenate([bq, bk]).reshape(4, P).T
        wv_c = np.zeros((D, 4 * 65), np.float32)
        bv_c = np.zeros((1, 4 * 65), np.float32)
        for h in range(4):
            wv_c[:, 65 * h : 65 * h + HD] = wv[:, HD * h : HD * (h + 1)]
            bv_c[0, 65 * h : 65 * h + HD] = bv[HD * h : HD * (h + 1)]
        wp_c = Wproj[256 * g : 256 * (g + 1), :]
        in_maps.append(
            {
                "xT": np.ascontiguousarray(x[b].T).astype(BF),
                "wqk": wqk_c.astype(BF),
                "bqk": np.ascontiguousarray(bqk_c, np.float32),
                "wv": wv_c.astype(BF),
                "bv": bv_c.astype(BF),
                "wp": np.ascontiguousarray(wp_c).astype(BF),
                "masks": masks.astype(BF),
            }
        )
    return in_maps


def kernel(x, Wqkv, bqkv, Wproj, bproj, _trace=False, _trace_out=None):
    from concourse.bass_utils import run_bass_kernel_spmd

    if "nc" not in _CACHE:
        _CACHE["nc"] = _build()
    nc = _CACHE["nc"]

    x = np.asarray(x, np.float32)
    Wqkv = np.asarray(Wqkv, np.float32)
    bqkv = np.asarray(bqkv, np.float32)
    Wproj = np.asarray(Wproj, np.float32)
    bproj = np.asarray(bproj, np.float32)

    in_maps = _prep_inputs(x, Wqkv, bqkv, Wproj)
    res = run_bass_kernel_spmd(
        nc, in_maps, core_ids=list(range(N_CORES)), trace=_trace
    )
    if _trace_out is not None:
        _trace_out.append(res)

    out = np.empty((B, T, D), np.float32)
    for b in range(B):
        acc = res.results[HPC * b]["out"].astype(np.float32)
        for g in range(1, HPC):
            acc = acc + res.results[HPC * b + g]["out"]
        out[b] = acc + bproj[None, :]
    return out


# revision 27
# speedup vs baseline: 1.6496x; 1.0217x over previous
"""Causal self-attention (B=2, T=2048, D=1024, H=16, hd=64) on 8 TRN2 cores.

Sharding: 2 batches x 4 head-groups (4 heads each). Each core computes the
full pipeline for its (batch, head-group); the host sums the 4 per-batch
partials (tensor-parallel reduce) and adds bproj.

v2 design (vs the f32r baseline):
 - bf16 everywhere on the PE (PSUM accumulation stays f32): q/k/v weights,
   activations, exp, proj. Validated ~3e-3 rel err vs the 2e-2 gate.
 - v is produced directly in natural [token, hd] layout (stationary = xT
   block, moving = Wv columns), with the +bv handled by a K=1 ones-row
   matmul. No PE transposes.
 - Attention runs head-pair-sequential per q-slice so only one y-accumulator
   psum tile is live at a time; its pool slot rotates per head-pair, which
   removes the slice-boundary stalls the baseline had.
 - Causal diagonal blocks slice the St/exp/y ranges instead of memsetting
   masked regions (bf16 matmuls run 1 cycle/row at any width).
 - Softmax normalization: DVE copies the two denominator rows out of PSUM,
   a K=1 bf16 matmul broadcasts them across 64 partitions, one
   reciprocal_approx_fast inverts both broadcasts, and two DVE muls write
   the normalized yT with the second head shifted to partitions 64:127 so
   the output projection contracts K=128 (two matmuls instead of four).
"""

import sys

sys.path.insert(0, "/opt/trn_rl_repo")

import numpy as np
import ml_dtypes
from collections import deque

B, T, D = 2, 2048, 1024
N_HEAD = 16
HD = 64  # head dim
HPC = 4  # heads per core
N_CORES = 8

P = 128
NJ = 512  # q-slice width
JT = T // NJ  # 4 q-slices
KT = D // P  # 8 contraction tiles
IT = T // P  # 16 token tiles

_CACHE = {}


def _build():
    import concourse.bass as bass  # noqa: F401
    import concourse.mybir as mybir
    import concourse.tile as tile
    from concourse import bacc

    F32 = mybir.dt.float32
    BF16 = mybir.dt.bfloat16
    AF = mybir.ActivationFunctionType

    nc = bacc.Bacc(None, target_bir_lowering=False)
    xT_d = nc.dram_tensor("xT", [D, T], BF16, kind="ExternalInput")
    wqk_d = nc.dram_tensor("wqk", [D, 4 * P], BF16, kind="ExternalInput")
    bqk_d = nc.dram_tensor("bqk", [P, 4], F32, kind="ExternalInput")
    wv_d = nc.dram_tensor("wv", [D, 4 * 65], BF16, kind="ExternalInput")
    bv_d = nc.dram_tensor("bv", [1, 4 * 65], BF16, kind="ExternalInput")
    wp_d = nc.dram_tensor("wp", [2 * P, D], BF16, kind="ExternalInput")
    masks_d = nc.dram_tensor("masks", [P, 2 * P], BF16, kind="ExternalInput")
    out_d = nc.dram_tensor("out", [T, D], F32, kind="ExternalOutput")

    with tile.TileContext(nc) as tc:
        with (
            tc.tile_pool(name="const", bufs=1) as const,
            tc.tile_pool(name="stp", bufs=2, space="PSUM") as stp,
            tc.tile_pool(name="yp", bufs=2, space="PSUM") as yp,
            tc.tile_pool(name="expp", bufs=7) as expp,
            tc.tile_pool(name="denp", bufs=2) as denp,
            tc.tile_pool(name="recp", bufs=2) as recp,
            tc.tile_pool(name="outp", bufs=2) as outp,
        ):
            w_sb = const.tile([P, KT, 4 * P], BF16)
            bqk_sb = const.tile([P, 4], F32)
            wv_sb = const.tile([P, KT, 4, 65], BF16)
            bv_sb = const.tile([P, 4, 65], BF16)
            wp_sb = const.tile([P, 2, D], BF16)
            masks_sb = const.tile([P, 2, P], BF16)
            ones_sb = const.tile([P, P], BF16)
            xt_sb = const.tile([P, KT, T], BF16)
            qkvT = const.tile([P, 4, T], BF16)
            vnat = const.tile([P, IT, 4, 65], BF16)
            yt2 = const.tile([P, 2, T], BF16)

            nc.gpsimd.memset(ones_sb[:], 1.0)
            nc.gpsimd.memset(vnat[:], 1.0)

            xT_r = xT_d.rearrange("(kt p) t -> p kt t", p=P)
            wqk_r = wqk_d.rearrange("(kt p) n -> p kt n", p=P)
            wv_r = wv_d.rearrange("(kt p) n -> p kt n", p=P)
            # Merged multi-dim DMAs: the sync queue issues each PSEUDO_DMA in
            # ~600ns, so per-k transfers serialize for ~16us; merged ones
            # issue once and let the DMA engine stream.
            nc.sync.dma_start(bqk_sb[:], bqk_d[:])
            for k0, k1 in ((0, 2), (2, 5), (5, 8)):
                nc.sync.dma_start(w_sb[:, k0:k1, :], wqk_r[:, k0:k1, :])
                nc.sync.dma_start(xt_sb[:, k0:k1, 0:NJ], xT_r[:, k0:k1, 0:NJ])
            nc.sync.dma_start(wv_sb[:, :, :, :], wv_r[:, :, :])
            nc.sync.dma_start(bv_sb[0:1, :, :], bv_d[:])
            nc.sync.dma_start(
                wp_sb[:], wp_d.rearrange("(hp p) d -> p hp d", p=P)
            )
            nc.sync.dma_start(masks_sb[:], masks_d.rearrange("p (a b) -> p a b", a=2))

            def emit_xt(j):
                # One multi-dim DMA per slice: 8x fewer sync-queue issue slots.
                nc.sync.dma_start(
                    xt_sb[:, :, j * NJ : (j + 1) * NJ],
                    xT_r[:, :, j * NJ : (j + 1) * NJ],
                )

            # ---- fills: qkv q/k groups + natural-layout v tiles ------------
            def emit_qkv(j, m):
                ps = stp.tile([P, NJ], F32, tag="st", name=f"qkvps{j}_{m}")
                for k in range(KT):
                    nc.tensor.matmul(
                        ps[:],
                        w_sb[:, k, m * P : (m + 1) * P],
                        xt_sb[:, k, j * NJ : (j + 1) * NJ],
                        start=(k == 0),
                        stop=(k == KT - 1),
                    )
                with nc.allow_low_precision(reason="bf16 activations"):
                    nc.vector.tensor_scalar_add(
                        qkvT[:, m, j * NJ : (j + 1) * NJ], ps[:], bqk_sb[:, m : m + 1]
                    )

            def emit_v(ii):
                ps = stp.tile([P, 4, 65], F32, tag="st", name=f"vps{ii}")
                for k in range(KT):
                    nc.tensor.matmul(
                        ps[:],
                        xt_sb[:, k, ii * P : (ii + 1) * P],
                        wv_sb[:, k, :, :],
                        start=(k == 0),
                        stop=False,
                    )
                nc.tensor.matmul(
                    ps[:],
                    ones_sb[0:1, 0:P],
                    bv_sb[0:1, :, :],
                    start=False,
                    stop=True,
                )
                with nc.allow_low_precision(reason="bf16 activations"):
                    nc.vector.tensor_copy(vnat[:, ii, :, 0:HD], ps[:, :, 0:HD])

            fill_q = deque()

            def push_fill(j):
                fill_q.append(("qkv", j, 0))
                fill_q.append(("qkv", j, 2))
                for ii in range(4 * j, 4 * j + 4):
                    fill_q.append(("v", ii))
                fill_q.append(("qkv", j, 1))
                fill_q.append(("qkv", j, 3))

            def emit_fill(item):
                if item[0] == "qkv":
                    emit_qkv(item[1], item[2])
                else:
                    emit_v(item[1])

            proj_q = deque()  # (qm, n, min_j)

            def emit_proj(item, act_copy=False):
                # po lives in the "st" ring: every st-slot tenant's releasing
                # reader is emitted in the same emit_* call, so a PE matmul
                # here can never wait on a not-yet-emitted instruction.
                qm, n = item[0], item[1]
                po = stp.tile([P, NJ], F32, tag="st", name=f"po{qm}_{n}")
                for hp in range(2):
                    nc.tensor.matmul(
                        po[:],
                        yt2[:, hp, qm * P : (qm + 1) * P],
                        wp_sb[:, hp, n * NJ : (n + 1) * NJ],
                        start=(hp == 0),
                        stop=(hp == 1),
                    )
                ot = outp.tile([P, NJ], F32, tag="ot")
                if act_copy:
                    nc.scalar.copy(ot[:], po[:])
                else:
                    nc.vector.tensor_copy(ot[:], po[:])
                nc.sync.dma_start(
                    out_d[qm * P : (qm + 1) * P, n * NJ : (n + 1) * NJ], ot[:]
                )

            cur_j = [0]

            def pump():
                tick[0] += 1
                if norm_q and norm_q[0][0] <= tick[0]:
                    norm_q.popleft()[1]()
                if fill_q:
                    emit_fill(fill_q.popleft())
                elif (
                    proj_q
                    and proj_q[0][2] <= cur_j[0]
                    and tick[0] % 2 == 0
                ):
                    # Pace projections: they are the only PE filler left in
                    # late slices, whose attention stream is exp-throughput
                    # bound; spread them 1-per-2 iterations.
                    emit_proj(proj_q.popleft())

            # ---- attention -------------------------------------------------
            def emit_st_exp(j, hp, i):
                r = i - 4 * j
                c0 = max(0, P * r)
                st = stp.tile([P, 2, NJ], F32, tag="st", name=f"st{j}_{hp}_{i}")
                for par in range(2):
                    rows = slice(HD * par, HD * par + HD)
                    nc.tensor.matmul(
                        st[:, par, c0:NJ],
                        qkvT[rows, 2 + hp, i * P : (i + 1) * P],
                        qkvT[rows, hp, j * NJ + c0 : (j + 1) * NJ],
                        start=True,
                        stop=True,
                        tile_position=(HD * par, 0),
                    )
                exp2 = expp.tile([P, 2, NJ], BF16, tag="exp")
                nc.scalar.activation(exp2[:, :, c0:NJ], st[:, :, c0:NJ], AF.Exp)
                if r >= 0:
                    nc.gpsimd.tensor_mul(
                        exp2[:, :, c0 : c0 + P],
                        exp2[:, :, c0 : c0 + P],
                        masks_sb[:],
                    )
                return exp2

            def emit_y(j, hp, i, exp2, y2, last):
                r = i - 4 * j
                c0 = max(0, P * r)
                for par in range(2):
                    nc.tensor.matmul(
                        y2[0:65, par, c0:NJ],
                        vnat[:, i, 2 * hp + par, :],
                        exp2[:, par, c0:NJ],
                        start=(i == 0),
                        stop=last,
                    )

            def pop_y():
                j_, hp_, i_, exp2_, y2_, last_ = y_q.popleft()
                emit_y(j_, hp_, i_, exp2_, y2_, last_)
                if last_:
                    emit_norm_stages(hp_, j_, y2_)

            # Norm chain is emitted in stages pumped between later iterations
            # so its PE matmuls never block the St stream while the DVE den
            # copy / reciprocal latency drains. Each stage carries a due tick:
            # the PE queue is in-order, so a bc matmul emitted too early still
            # stalls the engine behind the 1.2us den copy.
            norm_q = deque()  # (due_tick, closure)
            tick = [0]

            def emit_norm_stages(hp, j, y2):
                # At most one head-pair's stages may be pending: drain the
                # previous ones so y-slot release ops are always emitted
                # before the slot's next-next tenant allocates.
                while norm_q:
                    norm_q.popleft()[1]()
                state = {}

                def s_den():
                    den = denp.tile([P, 2, NJ], BF16, tag="den")
                    with nc.allow_low_precision(reason="bf16 denominator"):
                        nc.vector.tensor_copy(den[HD:65, :, :], y2[HD:65, :, :])
                    state["den"] = den

                def s_bc():
                    bc = stp.tile([P, 2, NJ], F32, tag="st", name=f"bc{hp}_{j}")
                    den = state["den"]
                    for par in range(2):
                        nc.tensor.matmul(
                            bc[0:HD, par, :],
                            ones_sb[HD : HD + 1, 0:HD],
                            den[HD : HD + 1, par, :],
                            start=True,
                            stop=True,
                        )
                    state["bc"] = bc

                def s_rec():
                    rec = recp.tile([P, 2, NJ], F32, tag="rec")
                    nc.vector.reciprocal_approx_fast(
                        rec[0:HD, :, :], state["bc"][0:HD, :, :]
                    )
                    state["rec"] = rec

                def s_mul():
                    rec = state["rec"]
                    with nc.allow_low_precision(reason="bf16 yT"):
                        for par in range(2):
                            nc.vector.tensor_mul(
                                yt2[
                                    HD * par : HD * par + HD,
                                    hp,
                                    j * NJ : (j + 1) * NJ,
                                ],
                                y2[0:HD, par, :],
                                rec[0:HD, par, :],
                            )
                    if hp == 1:
                        # Projections read yt2 slice j; queue them only once
                        # both head-pairs' norm muls are emitted (Tile deps
                        # follow emission order). Held until slice j+2 where
                        # the exp-bound attention stream needs PE filler.
                        mj = min(j + 2, JT - 1)
                        for qm in range(4 * j, 4 * j + 4):
                            proj_q.append((qm, 0, mj))
                            proj_q.append((qm, 1, mj))

                # den copy only waits on y2's stop; emit it immediately so the
                # chain starts draining, then pump the rest with spacing.
                s_den()
                t = tick[0]
                norm_q.append((t + 3, s_bc))
                norm_q.append((t + 5, s_rec))
                norm_q.append((t + 6, s_mul))

            DEPTH = 4
            y_q = deque()  # (j, hp, i, exp2, y2, last) — trails across hp

            push_fill(0)
            while fill_q:
                emit_fill(fill_q.popleft())

            for j in range(JT):
                cur_j[0] = j
                if j + 1 < JT:
                    emit_xt(j + 1)
                    push_fill(j + 1)
                n_i = 4 * j + 4
                for hp in range(2):
                    # At hp start norm_q can only hold stages from two
                    # head-pairs ago (the previous hp's aren't pushed yet):
                    # drain them so this hp's y2 slot releases are emitted
                    # before its first write.
                    while norm_q:
                        norm_q.popleft()[1]()
                    y2 = yp.tile([P, 2, NJ], F32, tag="y", name=f"y2_{hp}_{j}")
                    for i in range(n_i):
                        if len(y_q) > DEPTH:
                            pop_y()
                        exp2 = emit_st_exp(j, hp, i)
                        y_q.append((j, hp, i, exp2, y2, i == n_i - 1))
                        pump()

            while y_q:
                pop_y()
            while norm_q:
                norm_q.popleft()[1]()
            while fill_q:
                emit_fill(fill_q.popleft())
            # Tail: exp stream is done, ACT is free — alternate the out
            # copies between ACT and DVE so they pipeline.
            for idx in range(len(proj_q)):
                emit_proj(proj_q.popleft(), act_copy=(idx % 2 == 0))

    nc.compile()
    return nc


def _prep_inputs(x, Wqkv, bqkv, Wproj):
    """Per-core input maps. Core c -> batch c//4, heads 4*(c%4) .. +4."""
    BF = ml_dtypes.bfloat16
    scale = np.float32(1.0 / np.sqrt(HD))
    pp = np.arange(P)[:, None]
    ff = np.arange(P)[None, :]
    tri = (ff >= pp).astype(np.float32)
    masks = np.concatenate([tri, tri], axis=1)

    in_maps = []
    for c in range(N_CORES):
        b, g = divmod(c, HPC)
        cs = slice(256 * g, 256 * g + 256)
        wq = Wqkv[:, 0 * D :][:, cs] * scale
        wk = Wqkv[:, 1 * D : 2 * D][:, cs]
        wv = Wqkv[:, 2 * D : 3 * D][:, cs]
        wqk_c = np.ascontiguousarray(np.concatenate([wq, wk], axis=1))
        bq = bqkv[0 * D :][cs] * scale
        bk = bqkv[1 * D : 2 * D][cs]
        bv = bqkv[2 * D : 3 * D][cs]
        bqk_c = np.concatenate([bq, bk]).reshape(4, P).T
        wv_c = np.zeros((D, 4 * 65), np.float32)
        bv_c = np.zeros((1, 4 * 65), np.float32)
        for h in range(4):
            wv_c[:, 65 * h : 65 * h + HD] = wv[:, HD * h : HD * (h + 1)]
            bv_c[0, 65 * h : 65 * h + HD] = bv[HD * h : HD * (h + 1)]
        wp_c = Wproj[256 * g : 256 * (g + 1), :]
        in_maps.append(
            {
                "xT": np.ascontiguousarray(x[b].T).astype(BF),
                "wqk": wqk_c.astype(BF),
                "bqk": np.ascontiguousarray(bqk_c, np.float32),
                "wv": wv_c.astype(BF),
                "bv": bv_c.astype(BF),
                "wp": np.ascontiguousarray(wp_c).astype(BF),
                "masks": masks.astype(BF),
            }
        )
    return in_maps


def kernel(x, Wqkv, bqkv, Wproj, bproj, _trace=False, _trace_out=None):
    from concourse.bass_utils import run_bass_kernel_spmd

    if "nc" not in _CACHE:
        _CACHE["nc"] = _build()
    nc = _CACHE["nc"]

    x = np.asarray(x, np.float32)
    Wqkv = np.asarray(Wqkv, np.float32)
    bqkv = np.asarray(bqkv, np.float32)
    Wproj = np.asarray(Wproj, np.float32)
    bproj = np.asarray(bproj, np.float32)

    in_maps = _prep_inputs(x, Wqkv, bqkv, Wproj)
    res = run_bass_kernel_spmd(
        nc, in_maps, core_ids=list(range(N_CORES)), trace=_trace
    )
    if _trace_out is not None:
        _trace_out.append(res)

    out = np.empty((B, T, D), np.float32)
    for b in range(B):
        acc = res.results[HPC * b]["out"].astype(np.float32)
        for g in range(1, HPC):
            acc = acc + res.results[HPC * b + g]["out"]
        out[b] = acc + bproj[None, :]
    return out
